# revision 46
# baseline (speedup 1.0000x reference)
"""BotRGCN forward pass on 8 Trainium2 NeuronCores (Bass/Tile SPMD).

Strategy (graph/data parallel, per the sharding hint):
  - Nodes are split into 128-row blocks; core k owns blocks
    [k*bpc, (k+1)*bpc).
  - Feature encoder: x is transposed/cast to bf16 on host (xT [1552, npc]
    per core); one fused matmul against a combined block-sparse
    W_enc [1552,128], then the w_in linear.  The encoder emits h in both
    orientations (hT for later matmuls, row-major h for the gather table);
    the row-major orientation is produced with the X-bar DMA transpose.
  - Per RGCN layer: AllGather the row-major bf16 h table (in C chunks so
    the collective overlaps the compute that produces its input) so every
    core holds all N rows; for each owned 128-dst block, indirect-DMA
    gather the source rows of its edges (host-sorted by dst block, padded
    to 128-edge tiles; one gather per tile — HW reads one offset per
    partition — spread over 2 SWDGE queues with a 2048-descriptor ring so
    ~16 gathers stay in flight against HBM random-read latency), build a
    scaled selection matrix
    S [128 edges, 256] = (iota256 == dst_local + 128*rel) * (1/cnt)
    on DVE, and matmul-accumulate both relations' partial sums
    P^T [feat, 2*128 dst] in PSUM.  Blocks are processed SB at a time so
    the transform/root matmuls run SB*128 wide and DMAs are batched.
    Both layers' transposed activation tables stay resident in SBUF
    (2 x 57.6KB/partition), removing hT DRAM traffic from the gather
    windows.
    out^T = W_r^T P_r^T + root^T h^T (+bias, LeakyReLU) stays in the
    transposed orientation.  The classifier is fused into layer 2; the
    final output is produced transposed [128, npc] bf16 and
    transposed/cast back on host.
  - The h_full gather table uses a chunk-major row order
    (chunk, core, row) so each AllGather chunk's output is contiguous.

kernel() takes FULL inputs and returns the FULL output.
"""

import math
import os as _os
import sys
from contextlib import ExitStack

sys.path.insert(0, "/opt/trn_rl_repo")

import ml_dtypes
import numpy as np

import concourse.bass as bass
import concourse.bacc as bacc_mod
import concourse.tile as tile
from concourse import mybir
from concourse.bass_utils import run_bass_kernel_spmd
from concourse.masks import make_identity

NCORES = 8
P = 128
R = 2
SB = 3                   # dst blocks per super-block (PSUM budget: 8 banks)
GSTRAT = _os.environ.get("BOT_GSTRAT", "tile")  # "tile" | "dg"
C = int(_os.environ.get("BOT_C", "1" if GSTRAT == "dg" else "3"))
GCAP = int(_os.environ.get("BOT_GCAP", "1"))   # tiles per indirect call (tile)
GB = 3 * SB              # blocks per dma_gather group (dg)
NQ = int(_os.environ.get("BOT_NQ", "2"))    # SWDGE queues for gathers
GBUFS = int(_os.environ.get("BOT_GBUFS", "3"))  # gather tile pool depth
TRP = _os.environ.get("BOT_TRP", "pe")      # "dma" (xbar) | "pe" transpose
# SWDGE descriptor carveout (bytes/partition); ring depth = SCRATCH/16 descs
# (2048 descs = 16 outstanding 128-row gathers; the gather chain is
# random-read-latency bound, so ring depth sets the achieved rate)
SCRATCH = int(_os.environ.get("BOT_SCRATCH", "32768"))
HTRES = int(_os.environ.get("BOT_HTRES", "1"))  # keep hT tables in SBUF
SW = SB * P              # encoder sweep width
D_IN = 1552
HID = 128
D_NUM, D_TWEET, D_CAT, D_DES = 5, 768, 11, 768

# flow dtype for activations / gather tables / matmul operands
if _os.environ.get("BOT_FLOW", "bf16") == "f32":
    FLOW_DT = mybir.dt.float32
    FLOW_NP = np.float32
else:
    FLOW_DT = mybir.dt.bfloat16
    FLOW_NP = ml_dtypes.bfloat16

TRACE = False
LAST_RESULTS = None
TIME_RUNS = int(_os.environ.get("BOT_TIME_RUNS", "0"))
LAST_TIME_NS = None
LAST_TIMES = None
LAST_BURSTS = None

F32 = mybir.dt.float32
AF = mybir.ActivationFunctionType
ALU = mybir.AluOpType


def _host_prep(x, src, dst, et, weights):
    N = x.shape[0]
    E = src.shape[0]
    B_total = (N + P - 1) // P
    bpc = (B_total + NCORES - 1) // NCORES
    gran = SB * C
    bpc = gran * ((bpc + gran - 1) // gran)          # multiple of SB and C
    npc = bpc * P
    Npad = npc * NCORES
    rng = npc // C                                   # rows per AG chunk

    # ---- combined encoder weight ----
    w_des, b_des = weights["w_des"], weights["b_des"]
    w_tweet, b_tweet = weights["w_tweet"], weights["b_tweet"]
    w_num, b_num = weights["w_num"], weights["b_num"]
    w_cat, b_cat = weights["w_cat"], weights["b_cat"]
    SUB = w_des.shape[1]
    W_enc = np.zeros((D_IN, 4 * SUB), np.float32)
    o_num, o_tweet, o_cat, o_des = 0, D_NUM, D_NUM + D_TWEET, D_NUM + D_TWEET + D_CAT
    W_enc[o_des:o_des + D_DES, 0 * SUB:1 * SUB] = w_des
    W_enc[o_tweet:o_tweet + D_TWEET, 1 * SUB:2 * SUB] = w_tweet
    W_enc[o_num:o_num + D_NUM, 2 * SUB:3 * SUB] = w_num
    W_enc[o_cat:o_cat + D_CAT, 3 * SUB:4 * SUB] = w_cat
    b_enc = np.concatenate([b_des, b_tweet, b_num, b_cat]).astype(np.float32)

    # ---- per-core xT slices ----
    xTs = []
    for k in range(NCORES):
        lo, hi = k * npc, min((k + 1) * npc, N)
        xk = np.zeros((D_IN, npc), FLOW_NP)
        if hi > lo:
            xk[:, : hi - lo] = x[lo:hi].T.astype(FLOW_NP)
        xTs.append(xk)

    # ---- edge plan: group by dst block, both relations together ----
    scale = np.zeros(E, np.float32)
    for r in range(R):
        m = et == r
        cnt_r = np.bincount(dst[m], minlength=N)
        scale[m] = 1.0 / np.maximum(cnt_r[dst[m]], 1)

    gblk = (dst >> 7).astype(np.int64)              # global dst block
    counts = np.bincount(gblk, minlength=B_total)

    # tiles per local block: max over cores (identical SPMD program)
    T = np.zeros(bpc, np.int64)
    for lb in range(bpc):
        cmax = 0
        for k in range(NCORES):
            g = k * bpc + lb
            if g < B_total:
                cmax = max(cmax, counts[g])
        T[lb] = max(1, (cmax + P - 1) // P)
    tile_off = np.zeros(bpc, np.int64)
    tile_off[1:] = np.cumsum(T)[:-1]
    TILES = int(T.sum())

    # h_full row id for a source node: chunk-major (chunk, core, row)
    # so each AllGather chunk's output region is contiguous.
    def full_row(s):
        k = s // npc
        lr = s % npc
        return (lr // rng) * (NCORES * rng) + k * rng + (lr % rng)

    plan_src = np.zeros((NCORES, P, TILES), np.int32)
    plan_dst = np.full((NCORES, P, TILES), -1.0, np.float32)
    plan_scl = np.zeros((NCORES, P, TILES), np.float32)

    order = np.argsort(gblk, kind="stable")
    gs = gblk[order]
    starts = np.zeros(B_total + 1, np.int64)
    starts[1:] = np.cumsum(counts)
    rank = np.arange(E, dtype=np.int64) - starts[gs]
    kk = gs // bpc
    lb = gs % bpc
    tt = tile_off[lb] + (rank >> 7)
    pp = rank & 127
    plan_src[kk, pp, tt] = full_row(src[order]).astype(np.int32)
    plan_dst[kk, pp, tt] = ((dst[order] & 127) + P * et[order]).astype(np.float32)
    plan_scl[kk, pp, tt] = scale[order]

    prep = dict(
        N=N, B_total=B_total, bpc=bpc, npc=npc, Npad=Npad, rng=rng,
        W_enc=W_enc, b_enc=b_enc, xTs=xTs,
        T=T, tile_off=tile_off, TILES=TILES,
        plan_src=plan_src, plan_dst=plan_dst, plan_scl=plan_scl,
    )
    if GSTRAT == "dg":
        prep.update(_host_prep_dg(src, dst, et, scale, N, bpc, npc))
    return prep


def _host_prep_dg(src, dst, et, scale, N, bpc, npc):
    """dma_gather plan: bucket edges by source core; tiles are
    (dst-block, bucket)-pure; one dma_gather per (group of GB blocks, bucket).
    Requires C == 1 (h_full rows = node ids)."""
    assert C == 1
    E = src.shape[0]
    B_total = (N + P - 1) // P
    NB = NCORES                                  # buckets = source cores
    gblk = (dst >> 7).astype(np.int64)
    bu = src // npc                              # bucket
    cell = gblk * NB + bu                        # global (block, bucket) cell
    counts = np.bincount(cell, minlength=B_total * NB)

    # tiles per local (block, bucket): max over cores
    T2 = np.zeros((bpc, NB), np.int64)
    for k in range(NCORES):
        lo = k * bpc * NB
        cc = counts[lo:lo + bpc * NB]
        if len(cc) < bpc * NB:
            cc = np.pad(cc, (0, bpc * NB - len(cc)))
        T2 = np.maximum(T2, (cc.reshape(bpc, NB) + P - 1) // P)
    empty = T2.sum(1) == 0
    T2[empty, 0] = 1                             # ensure >=1 tile per block
    tid_off = np.zeros(bpc * NB + 1, np.int64)   # plan order: (block, bucket)
    tid_off[1:] = np.cumsum(T2.reshape(-1))
    TILES2 = int(tid_off[-1])

    # per-tile idx/dst/scl planes
    I2 = np.zeros((NCORES, P, TILES2), np.int16)
    D2 = np.full((NCORES, P, TILES2), -1.0, np.float32)
    S2 = np.zeros((NCORES, P, TILES2), np.float32)
    order = np.argsort(cell, kind="stable")
    cs = cell[order]
    starts = np.zeros(B_total * NB + 1, np.int64)
    starts[1:] = np.cumsum(counts)
    rank = np.arange(E, dtype=np.int64) - starts[cs]
    kk = cs // (bpc * NB)
    lcell = cs % (bpc * NB)                      # (lb * NB + u)
    tt = tid_off[lcell] + (rank >> 7)
    pp_ = rank & 127
    I2[kk, pp_, tt] = (src[order] % npc).astype(np.int16)
    D2[kk, pp_, tt] = ((dst[order] & 127) + P * et[order]).astype(np.float32)
    S2[kk, pp_, tt] = scale[order]

    # group/call structure: per (group gi of GB blocks, bucket u)
    NG = bpc // GB
    call_nidx = np.zeros((NG, NB), np.int64)
    for gi in range(NG):
        for u in range(NB):
            call_nidx[gi, u] = T2[gi * GB:(gi + 1) * GB, u].sum() * P
    # g-tile column of each tile, in call order (u-major within group)
    gcol = np.zeros(TILES2, np.int64)
    call_colstart = np.zeros((NG, NB), np.int64)  # idx col offsets (per 16)
    call_gbase = np.zeros((NG, NB), np.int64)     # g-tile column base
    gidx_cols = TILES2 * P // 16
    gidx = np.zeros((NCORES, P, gidx_cols), np.int16)
    blk_tiles = [[] for _ in range(bpc)]         # per block: [(tid, gcol)]
    col_off = 0                                  # idx column cursor
    for gi in range(NG):
        base = 0
        for u in range(NB):
            call_colstart[gi, u] = col_off
            call_gbase[gi, u] = base
            for lb in range(gi * GB, (gi + 1) * GB):
                t0 = int(tid_off[lb * NB + u])
                for t in range(int(T2[lb, u])):
                    tid = t0 + t
                    gcol[tid] = base
                    blk_tiles[lb].append((tid, base))
                    # idx wrap: tile's 128 slots -> 8 cols of 16, replicated
                    w = I2[:, :, tid].reshape(NCORES, 8, 16)  # [k, col, 16p]
                    for k in range(NCORES):
                        gidx[k, :, col_off:col_off + 8] = np.tile(
                            w[k].T, (NCORES, 1))
                    base += 1
                    col_off += 8
    TGMAX = int(max(call_nidx.sum(1)) // P)

    return dict(T2=T2, tid_off=tid_off, TILES2=TILES2,
                plan_dst2=D2, plan_scl2=S2, gidx=gidx,
                call_nidx=call_nidx, call_colstart=call_colstart,
                call_gbase=call_gbase, blk_tiles=blk_tiles, TGMAX=TGMAX,
                NG=NG)


def _build_program(prep, weights):
    bpc, npc, Npad, TILES = prep["bpc"], prep["npc"], prep["Npad"], prep["TILES"]
    rng = prep["rng"]
    T, tile_off = prep["T"], prep["tile_off"]
    KCH = (D_IN + P - 1) // P                       # 13 chunks; last is 16 rows
    KFULL = D_IN // P                               # 12 full chunks

    nc = bacc_mod.Bacc(num_devices=NCORES, dynamic_dma_scratch_size=SCRATCH,
                       num_swdge_queues=NQ)

    # ---- I/O ----
    xT_t = nc.dram_tensor("xT", [D_IN, npc], FLOW_DT, kind="ExternalInput")
    if GSTRAT == "dg":
        NT = prep["TILES2"]
        gidx_t = nc.dram_tensor("gidx", [P, prep["gidx"].shape[2]],
                                mybir.dt.int16, kind="ExternalInput")
        psrc_t = None
    else:
        NT = TILES
        gidx_t = None
        psrc_t = nc.dram_tensor("plan_src", [P, TILES], mybir.dt.int32,
                                kind="ExternalInput")
    pdst_t = nc.dram_tensor("plan_dst", [P, NT], F32, kind="ExternalInput")
    pscl_t = nc.dram_tensor("plan_scl", [P, NT], F32, kind="ExternalInput")
    out_t = nc.dram_tensor("outT", [P, npc], FLOW_DT, kind="ExternalOutput")

    # ---- internal DRAM ----
    h_rows = [[nc.dram_tensor(f"h{l}_rows{c}", [rng, HID], FLOW_DT)
               for c in range(C)] for l in range(2)]
    h_full = [nc.dram_tensor(f"h{l}_full", [Npad, HID], FLOW_DT, addr_space="Shared")
              for l in range(2)]
    hT_dram = (None if HTRES else
               [nc.dram_tensor(f"h{l}T", [HID, npc], FLOW_DT) for l in range(2)])

    # ---- constants ----
    wenc_pad = np.zeros((KCH * P, HID), FLOW_NP)
    wenc_pad[:D_IN] = prep["W_enc"].astype(FLOW_NP)
    wenc_c = nc.inline_tensor(wenc_pad, "wenc")
    benc_c = nc.inline_tensor(prep["b_enc"].reshape(HID, 1), "benc")
    win_c = nc.inline_tensor(weights["w_in"].astype(FLOW_NP), "win")
    bin_c = nc.inline_tensor(
        weights["b_in"].astype(np.float32).reshape(HID, 1), "bin")

    lw = []
    for l, (wname, rname, bname) in enumerate(
        [("rg1_w", "rg1_root", "rg1_b"), ("rg2_w", "rg2_root", "rg2_b")]
    ):
        w = weights[wname].astype(FLOW_NP)
        root = weights[rname].astype(FLOW_NP)
        b = weights[bname].astype(np.float32).reshape(HID, 1)
        lw.append(dict(
            w0=nc.inline_tensor(w[0], f"l{l}w0"),
            w1=nc.inline_tensor(w[1], f"l{l}w1"),
            root=nc.inline_tensor(root, f"l{l}root"),
            b=nc.inline_tensor(b, f"l{l}b"),
        ))
    wcls_c = nc.inline_tensor(weights["w_cls"].astype(FLOW_NP), "wcls")
    bcls_c = nc.inline_tensor(
        weights["b_cls"].astype(np.float32).reshape(HID, 1), "bcls")
    iota2_c = nc.inline_tensor(
        np.tile(np.arange(2 * P, dtype=np.float32), (P, 1)).astype(FLOW_NP),
        "iota2")

    # gather call plan: runs of <=GCAP tiles, aligned to super-block starts
    gcalls = []                                      # per SB: list of (t0, n)
    for b0 in range(0, bpc, SB):
        t0 = int(tile_off[b0])
        tsb = int(T[b0:b0 + SB].sum())
        calls = []
        while tsb > 0:
            n = min(tsb, GCAP)
            calls.append((t0, n))
            t0 += n
            tsb -= n
        gcalls.append(calls)
    TMAX = max(sum(n for _, n in calls) for calls in gcalls)

    with ExitStack() as ctx:
        tc = ctx.enter_context(tile.TileContext(
            nc, num_cores=NCORES, pool_alloc_mode="queue",
            trace_sim=bool(int(_os.environ.get("BOT_TRACE_SIM", "0")))))
        cp = ctx.enter_context(tc.tile_pool(name="const", bufs=1))

        wenc_sb = cp.tile([P, KCH * P], FLOW_DT)
        for k in range(KCH):
            pk = min(P, D_IN - k * P)
            nc.sync.dma_start(out=wenc_sb[:pk, k * P:(k + 1) * P],
                              in_=wenc_c[k * P:k * P + pk, :])
        benc_sb = cp.tile([P, 1], F32)
        nc.sync.dma_start(out=benc_sb[:], in_=benc_c[:, :])
        win_sb = cp.tile([P, P], FLOW_DT)
        nc.sync.dma_start(out=win_sb[:], in_=win_c[:, :])
        bin_sb = cp.tile([P, 1], F32)
        nc.sync.dma_start(out=bin_sb[:], in_=bin_c[:, :])
        lsb = []
        for l in range(2):
            d = {}
            for key in ("w0", "w1", "root"):
                t_ = cp.tile([P, P], FLOW_DT, tag=f"w_l{l}_{key}")
                nc.sync.dma_start(out=t_[:], in_=lw[l][key][:, :])
                d[key] = t_
            bt = cp.tile([P, 1], F32, tag=f"b_l{l}")
            nc.sync.dma_start(out=bt[:], in_=lw[l]["b"][:, :])
            d["b"] = bt
            lsb.append(d)
        wcls_sb = cp.tile([P, P], FLOW_DT)
        nc.sync.dma_start(out=wcls_sb[:], in_=wcls_c[:, :])
        bcls_sb = cp.tile([P, 1], F32)
        nc.sync.dma_start(out=bcls_sb[:], in_=bcls_c[:, :])
        iota2_sb = cp.tile([P, 2 * P], FLOW_DT)
        nc.sync.dma_start(out=iota2_sb[:], in_=iota2_c[:, :])

        ident_sb = None
        if TRP == "pe":
            ident_sb = cp.tile([P, P], FLOW_DT)
            make_identity(nc, ident_sb[:])

        # SBUF-resident transposed activation tables (both layers)
        hTres = None
        if HTRES:
            hTres = [cp.tile([P, npc], FLOW_DT, tag=f"hT{l}res",
                             name=f"hT{l}res") for l in range(2)]

        if GSTRAT == "dg":
            gidx_sb = cp.tile([P, prep["gidx"].shape[2]], mybir.dt.int16)
            nc.sync.dma_start(out=gidx_sb[:], in_=gidx_t[:, :])
            psrc_sb = None
        else:
            psrc_sb = cp.tile([P, TILES], mybir.dt.int32)
            nc.sync.dma_start(out=psrc_sb[:], in_=psrc_t[:, :])
        pdst_sb = cp.tile([P, NT], F32)
        nc.sync.dma_start(out=pdst_sb[:], in_=pdst_t[:, :])
        pscl_sb = cp.tile([P, NT], F32)
        nc.sync.dma_start(out=pscl_sb[:], in_=pscl_t[:, :])

        def all_gather(l, c):
            nc.gpsimd.collective_compute(
                "AllGather", ALU.bypass, replica_groups=[list(range(NCORES))],
                ins=[h_rows[l][c][:, :]],
                outs=[h_full[l][c * NCORES * rng:(c + 1) * NCORES * rng, :]])

        REPEAT = int(_os.environ.get("BOT_REPEAT", "1"))
        SKIP = set(_os.environ.get("BOT_SKIP", "").split(","))
        for _rep in range(REPEAT):
            if _rep > 0:
                tc.strict_bb_all_engine_barrier()

            # ================= encoder =================
            with (
                tc.tile_pool(name="enc_sb", bufs=2) as ep,
                tc.tile_pool(name="enc_out", bufs=2) as hp_pool,
                tc.tile_pool(name="enc_rows", bufs=3) as rp,
                tc.tile_pool(name="enc_ps", bufs=2, space="PSUM") as pp,
            ):
                for off in range(0, npc, SW):
                    w = SW
                    hpsum = pp.tile([P, SW], F32, tag="enc")
                    xall = ep.tile([P, KCH * SW], FLOW_DT, tag="xall")
                    # batched load: 12 full chunks in one DMA + the 16-row tail
                    nc.sync.dma_start(
                        out=xall[:, :].rearrange(
                            "p (k n) -> p k n", n=SW)[:, :KFULL, :w],
                        in_=xT_t[0:KFULL * P, off:off + w].rearrange(
                            "(k p) n -> p k n", p=P))
                    nc.sync.dma_start(
                        out=xall[:D_IN - KFULL * P,
                                 KFULL * SW:KFULL * SW + w],
                        in_=xT_t[KFULL * P:D_IN, off:off + w])
                    for k in range(KCH):
                        pk = min(P, D_IN - k * P)
                        nc.tensor.matmul(out=hpsum[:, :w],
                                         lhsT=wenc_sb[:pk, k * P:(k + 1) * P],
                                         rhs=xall[:pk, k * SW:k * SW + w],
                                         start=(k == 0), stop=(k == KCH - 1))
                    hs = hp_pool.tile([P, SW], FLOW_DT, tag="henc")
                    nc.scalar.activation(out=hs[:, :w], in_=hpsum[:, :w],
                                         func=AF.Lrelu, bias=benc_sb[:, :1],
                                         alpha=0.01)
                    h2psum = pp.tile([P, SW], F32, tag="enc2")
                    nc.tensor.matmul(out=h2psum[:, :w], lhsT=win_sb[:],
                                     rhs=hs[:, :w], start=True, stop=True)
                    if HTRES:
                        hs2 = hTres[0][:, off:off + w]
                        hs2_sub = lambda t: hTres[0][:, off + t * P:
                                                     off + (t + 1) * P]
                    else:
                        hs2t = hp_pool.tile([P, SW], FLOW_DT, tag="henc2")
                        hs2 = hs2t[:, :w]
                        hs2_sub = lambda t: hs2t[:, t * P:(t + 1) * P]
                    nc.scalar.activation(out=hs2, in_=h2psum[:, :w],
                                         func=AF.Lrelu, bias=bin_sb[:, :1],
                                         alpha=0.01)
                    if not HTRES:
                        nc.sync.dma_start(out=hT_dram[0][:, off:off + w],
                                          in_=hs2)
                    rows = rp.tile([P, SW], FLOW_DT, tag="rows")
                    if TRP == "dma":
                        nc.sync.dma_start_transpose(
                            out=rows[:, :].rearrange("p (b f) -> p b f", f=P),
                            in_=hs2)
                    else:
                        for t in range(SB):
                            tp = pp.tile([P, P], FLOW_DT, tag="tr")
                            nc.tensor.transpose(
                                out=tp[:], in_=hs2_sub(t),
                                identity=ident_sb[:])
                            nc.scalar.activation(
                                out=rows[:, t * P:(t + 1) * P], in_=tp[:],
                                func=AF.Copy)
                    c = off // rng
                    nc.sync.dma_start(
                        out=h_rows[0][c][off - c * rng:off - c * rng + w, :]
                        .rearrange("(b p) f -> p b f", p=P),
                        in_=rows[:, :w].rearrange("p (b f) -> p b f", f=P))
                    if "ag" not in SKIP and (off + w) % rng == 0:
                        all_gather(0, c)

            # ================= RGCN layers =================
            for l in ([], range(2))["layers" not in SKIP]:
                with (
                    tc.tile_pool(name=f"l{l}_g", bufs=GBUFS) as gp,
                    tc.tile_pool(name=f"l{l}_s", bufs=8) as sp,
                    tc.tile_pool(name=f"l{l}_m", bufs=4) as mp,
                    tc.tile_pool(name=f"l{l}_ps", bufs=2, space="PSUM") as pp,
                ):
                    g = None
                    for b0 in range(0, bpc, SB):
                        if GSTRAT == "dg":
                            if b0 % GB == 0:       # new gather group
                                gi = b0 // GB
                                g = gp.tile([P, prep["TGMAX"] * P], FLOW_DT,
                                            tag="g")
                                if "gathers" not in SKIP:
                                    for u in range(NCORES):
                                        n = int(prep["call_nidx"][gi][u])
                                        if n == 0:
                                            continue
                                        cb = int(prep["call_gbase"][gi][u])
                                        cs = int(prep["call_colstart"][gi][u])
                                        nc.gpsimd.dma_gather(
                                            out_ap=g[:, cb * P:cb * P + n]
                                            .rearrange("p (j f) -> p j f", f=P),
                                            in_ap=h_full[l][u * npc:
                                                            (u + 1) * npc, :],
                                            idxs_ap=gidx_sb[:, cs:cs + n // 16],
                                            num_idxs=n,
                                            num_idxs_reg=n,
                                            elem_size=P)
                        else:
                            tbase = int(tile_off[b0])
                            g = gp.tile([P, TMAX * P], FLOW_DT, tag="g")
                            if "gathers" not in SKIP:
                                for (t0, n) in gcalls[b0 // SB]:
                                    o = t0 - tbase
                                    inst = nc.gpsimd.indirect_dma_start(
                                        out=g[:, o * P:(o + n) * P],
                                        out_offset=None,
                                        in_=h_full[l][:, :],
                                        in_offset=bass.IndirectOffsetOnAxis(
                                            ap=psrc_sb[:, t0:t0 + n],
                                            axis=0))
                                    q = t0 % NQ
                                    if q:
                                        inst.ins.queue = f"qPoolDynamic{q}"
                        if HTRES:
                            hTb = hTres[l][:, b0 * P:(b0 + SB) * P]
                        else:
                            hTbt = mp.tile([P, SB * P], FLOW_DT, tag="hTb")
                            nc.sync.dma_start(
                                out=hTbt[:],
                                in_=hT_dram[l][:, b0 * P:(b0 + SB) * P])
                            hTb = hTbt[:, :]
                        # edge aggregation: P3^T [feat, SB*2*128] in PSUM
                        P3 = pp.tile([P, SB * 2 * P], F32, tag="P3")
                        for bi in range(SB):
                            b = b0 + bi
                            if GSTRAT == "dg":
                                tl = prep["blk_tiles"][b]
                                for j, (tid, gc) in enumerate(tl):
                                    S = sp.tile([P, 2 * P], FLOW_DT, tag="s")
                                    nc.vector.tensor_scalar(
                                        out=S[:], in0=iota2_sb[:],
                                        scalar1=pdst_sb[:, tid:tid + 1],
                                        scalar2=pscl_sb[:, tid:tid + 1],
                                        op0=ALU.is_equal, op1=ALU.mult)
                                    nc.tensor.matmul(
                                        out=P3[:, bi * 2 * P:(bi + 1) * 2 * P],
                                        lhsT=g[:, gc * P:(gc + 1) * P],
                                        rhs=S[:],
                                        start=(j == 0), stop=(j == len(tl) - 1))
                            else:
                                Tb = int(T[b])
                                t0 = int(tile_off[b])
                                for t in range(Tb):
                                    lt = t0 - tbase + t
                                    S = sp.tile([P, 2 * P], FLOW_DT, tag="s")
                                    nc.vector.tensor_scalar(
                                        out=S[:], in0=iota2_sb[:],
                                        scalar1=pdst_sb[:, t0 + t:t0 + t + 1],
                                        scalar2=pscl_sb[:, t0 + t:t0 + t + 1],
                                        op0=ALU.is_equal, op1=ALU.mult)
                                    nc.tensor.matmul(
                                        out=P3[:, bi * 2 * P:(bi + 1) * 2 * P],
                                        lhsT=g[:, lt * P:(lt + 1) * P],
                                        rhs=S[:],
                                        start=(t == 0), stop=(t == Tb - 1))
                        Ps = mp.tile([P, SB * 2 * P], FLOW_DT, tag="ps")
                        nc.scalar.activation(out=Ps[:], in_=P3[:], func=AF.Copy)

                        op_ = pp.tile([P, SB * P], F32, tag="out")
                        for r in range(R):
                            rhs = Ps[:, :].rearrange(
                                "p (b c) -> p b c", c=2 * P)[:, :, r * P:(r + 1) * P]
                            nc.tensor.matmul(out=op_[:],
                                             lhsT=lsb[l][f"w{r}"][:], rhs=rhs,
                                             start=(r == 0), stop=False)
                        nc.tensor.matmul(out=op_[:], lhsT=lsb[l]["root"][:],
                                         rhs=hTb, start=False, stop=True)
                        if l == 0 and HTRES:
                            ho = hTres[1][:, b0 * P:(b0 + SB) * P]
                            ho_sub = lambda t: hTres[1][:, (b0 + t) * P:
                                                        (b0 + t + 1) * P]
                        else:
                            hot = mp.tile([P, SB * P], FLOW_DT, tag="ho")
                            ho = hot[:, :]
                            ho_sub = lambda t: hot[:, t * P:(t + 1) * P]
                        nc.scalar.activation(out=ho, in_=op_[:],
                                             func=AF.Lrelu,
                                             bias=lsb[l]["b"][:, :1], alpha=0.01)
                        if l == 0:
                            if not HTRES:
                                nc.sync.dma_start(
                                    out=hT_dram[1][:, b0 * P:(b0 + SB) * P],
                                    in_=ho)
                            rows = mp.tile([P, SB * P], FLOW_DT, tag="rows")
                            if TRP == "dma":
                                nc.sync.dma_start_transpose(
                                    out=rows[:, :].rearrange(
                                        "p (b f) -> p b f", f=P),
                                    in_=ho)
                            else:
                                for t in range(SB):
                                    tp = pp.tile([P, P], FLOW_DT, tag="tr2")
                                    nc.tensor.transpose(
                                        out=tp[:], in_=ho_sub(t),
                                        identity=ident_sb[:])
                                    nc.scalar.activation(
                                        out=rows[:, t * P:(t + 1) * P],
                                        in_=tp[:], func=AF.Copy)
                            off = b0 * P
                            c = off // rng
                            nc.sync.dma_start(
                                out=h_rows[1][c][off - c * rng:
                                                 off - c * rng + SB * P, :]
                                .rearrange("(b p) f -> p b f", p=P),
                                in_=rows[:, :].rearrange("p (b f) -> p b f", f=P))
                            if "ag" not in SKIP and (off + SB * P) % rng == 0:
                                all_gather(1, c)
                        else:
                            cpsum = pp.tile([P, SB * P], F32, tag="cls")
                            nc.tensor.matmul(out=cpsum[:], lhsT=wcls_sb[:],
                                             rhs=ho, start=True, stop=True)
                            osb = mp.tile([P, SB * P], FLOW_DT, tag="osb")
                            nc.scalar.activation(out=osb[:], in_=cpsum[:],
                                                 func=AF.Identity,
                                                 bias=bcls_sb[:, :1])
                            nc.sync.dma_start(
                                out=out_t[:, b0 * P:(b0 + SB) * P], in_=osb[:])

    if not nc.is_finalized():
        nc.finalize()
    return nc


def make_in_maps(prep):
    in_maps = []
    for k in range(NCORES):
        if GSTRAT == "dg":
            in_maps.append({
                "xT": prep["xTs"][k],
                "gidx": prep["gidx"][k],
                "plan_dst": prep["plan_dst2"][k],
                "plan_scl": prep["plan_scl2"][k],
            })
        else:
            in_maps.append({
                "xT": prep["xTs"][k],
                "plan_src": prep["plan_src"][k],
                "plan_dst": prep["plan_dst"][k],
                "plan_scl": prep["plan_scl"][k],
            })
    return in_maps


def kernel(**inputs):
    global LAST_RESULTS
    x = np.asarray(inputs["x"], np.float32)
    ei = np.asarray(inputs["edge_index"])
    et = np.asarray(inputs["edge_type"]).astype(np.int64)
    src = ei[0].astype(np.int64)
    dst = ei[1].astype(np.int64)

    weights = {k: np.asarray(v, np.float32) for k, v in inputs.items()
               if k not in ("x", "edge_index", "edge_type")}

    prep = _host_prep(x, src, dst, et, weights)
    nc = _build_program(prep, weights)

    in_maps = make_in_maps(prep)

    if TIME_RUNS > 0:
        results = _run_and_time(nc, in_maps, TIME_RUNS)
    else:
        res = run_bass_kernel_spmd(nc, in_maps, list(range(NCORES)), trace=TRACE)
        LAST_RESULTS = res
        results = res.results

    outs = [np.asarray(results[k]["outT"]).astype(np.float32).T
            for k in range(NCORES)]
    out = np.concatenate(outs, axis=0)[: prep["N"]]
    return np.ascontiguousarray(out, dtype=np.float32)


def _run_and_time(nc, in_maps, n_runs):
    """Mirror bass2jax.run_bass_via_pjrt's multi-core path, but jit once,
    pre-place inputs on the device mesh, and wall-clock repeated executes."""
    global LAST_TIME_NS, LAST_TIMES
    import time as _time
    import jax
    from jax.sharding import Mesh, PartitionSpec, NamedSharding
    from jax.experimental.shard_map import shard_map
    from concourse import bass2jax, mybir as _mb
    bass2jax.install_neuronx_cc_hook()

    partition_name = nc.partition_id_tensor.name if nc.partition_id_tensor else None
    in_names, out_names, out_avals, zero_outs = [], [], [], []
    for alloc in nc.m.functions[0].allocations:
        if not isinstance(alloc, _mb.MemoryLocationSet):
            continue
        name = alloc.memorylocations[0].name
        if alloc.kind == "ExternalInput":
            if name != partition_name:
                in_names.append(name)
        elif alloc.kind == "ExternalOutput":
            shape = tuple(alloc.tensor_shape)
            dtype = _mb.dt.np(alloc.dtype)
            out_names.append(name)
            out_avals.append(jax.core.ShapedArray(shape, dtype))
            zero_outs.append(np.zeros(shape, dtype))
    n_params = len(in_names)
    in_names = in_names + out_names
    if partition_name is not None:
        in_names.append(partition_name)

    def _body(*args):
        operands = list(args)
        if partition_name is not None:
            operands.append(bass2jax.partition_id_tensor())
        outs = bass2jax._bass_exec_p.bind(
            *operands,
            out_avals=tuple(out_avals),
            in_names=tuple(in_names),
            out_names=tuple(out_names),
            lowering_input_output_aliases=(),
            sim_require_finite=True,
            sim_require_nnan=True,
            nc=nc,
        )
        return tuple(outs)

    devices = jax.devices()[:NCORES]
    mesh = Mesh(np.asarray(devices), ("core",))
    n_outs = len(out_names)
    in_specs = (PartitionSpec("core"),) * (n_params + n_outs)
    out_specs = (PartitionSpec("core"),) * n_outs
    sharded = jax.jit(
        shard_map(_body, mesh=mesh, in_specs=in_specs, out_specs=out_specs,
                  check_rep=False),
        keep_unused=True,
    )
    per_core = [[np.asarray(m[name]) for name in in_names[:n_params]]
                for m in in_maps]
    sh = NamedSharding(mesh, PartitionSpec("core"))
    concat_in = [
        jax.device_put(
            np.concatenate([per_core[c][i] for c in range(NCORES)], axis=0), sh)
        for i in range(n_params)
    ]
    concat_zeros = [
        jax.device_put(np.zeros((NCORES * z.shape[0], *z.shape[1:]), z.dtype), sh)
        for z in zero_outs
    ]
    jax.block_until_ready(concat_in)
    jax.block_until_ready(concat_zeros)

    times = []
    out_arrs = None
    for i in range(max(2, n_runs)):
        t0 = _time.perf_counter()
        out_arrs = sharded(*concat_in, *concat_zeros)
        jax.block_until_ready(out_arrs)
        times.append(_time.perf_counter() - t0)
    # burst mode: pipeline B dispatches, amortizing relay latency
    B = int(_os.environ.get("BOT_BURST", "8"))
    bursts = []
    for _ in range(int(_os.environ.get("BOT_NBURST", "4"))):
        t0 = _time.perf_counter()
        outs = [sharded(*concat_in, *concat_zeros) for _ in range(B)]
        jax.block_until_ready(outs[-1])
        bursts.append((_time.perf_counter() - t0) / B)
    global LAST_BURSTS
    LAST_BURSTS = bursts
    LAST_TIMES = times
    LAST_TIME_NS = int(min(times[1:]) * 1e9)
    return [
        {name: np.asarray(out_arrs[i]).reshape(NCORES, *out_avals[i].shape)[c]
         for i, name in enumerate(out_names)}
        for c in range(NCORES)
    ]
